# revision 28
# baseline (speedup 1.0000x reference)
"""DIN-style attention unit (dense MLP scorer + masked softmax + weighted sum)
on 8 Trainium2 NeuronCores, data-parallel over the batch dimension.

kernel(**inputs) takes the full unsharded inputs and returns (out, att):
  out: (2048, 128) f32, att: (2048, 200) f32
"""
import sys, types, os
sys.path.insert(0, "/opt/trn_rl_repo")
import numpy as np

# --- antenv.axon_hooks shim so run_bass_kernel_spmd(trace=True) can profile ---
try:
    import antenv.axon_hooks  # noqa: F401
except Exception:
    try:
        from trn_agent_boot.trn_boot import _ntff_profile_via_ctypes
        _hook = _ntff_profile_via_ctypes('/opt/axon/libaxon_pjrt.so')
    except Exception:
        _hook = None
    _m = types.ModuleType('antenv.axon_hooks')
    _m.get_axon_ntff_profile_hook = lambda: _hook
    _m.set_axon_ntff_profile_hook = lambda h: None
    sys.modules['antenv.axon_hooks'] = _m

import concourse.bass as bass
import concourse.tile as tile
from concourse import bacc, mybir
import concourse.bass_utils as bass_utils
from concourse import masks

bass_utils.upload_artifacts = lambda tmpdir: "local://" + str(tmpdir)

FP32 = mybir.dt.float32
BF16 = mybir.dt.bfloat16
F32R = mybir.dt.float32r
U8 = mybir.dt.uint8
I32 = mybir.dt.int32
AF = mybir.ActivationFunctionType
ALU = mybir.AluOpType

N_CORES = 8
B, L, D, A = 2048, 200, 128, 64
BC = B // N_CORES          # 256 batch rows per core
L1C = L - 128              # second l-chunk: 72
FILLS = BC // 16           # 16 bank-fills of 16 batch rows
NEG_BIG = -1e30


def build_graph(alpha1: float, alpha2: float, slot_N):
    nc = bacc.Bacc("TRN2", target_bir_lowering=False, debug=False,
                   num_devices=N_CORES)

    keys_d = nc.dram_tensor("keys", [BC, L, D], FP32, kind="ExternalInput").ap()
    query_d = nc.dram_tensor("query", [BC, D], FP32, kind="ExternalInput").ap()
    lens_d = nc.dram_tensor("lens", [BC], FP32, kind="ExternalInput").ap()
    W1_d = nc.dram_tensor("W1", [4 * D, A], FP32, kind="ExternalInput").ap()
    b1_d = nc.dram_tensor("b1", [A], FP32, kind="ExternalInput").ap()
    W2_d = nc.dram_tensor("W2", [A, A], FP32, kind="ExternalInput").ap()
    b2_d = nc.dram_tensor("b2", [A], FP32, kind="ExternalInput").ap()
    W3_d = nc.dram_tensor("W3", [A, 1], FP32, kind="ExternalInput").ap()
    out_d = nc.dram_tensor("out", [FILLS, 128, 512], FP32,
                           kind="ExternalOutput").ap()
    att_d = nc.dram_tensor("att", [FILLS, 128, 400], FP32,
                           kind="ExternalOutput").ap()

    with tile.TileContext(nc) as tc:
        import contextlib
        ctx = contextlib.ExitStack()
        with ctx:
            cst = ctx.enter_context(tc.tile_pool(name="cst", bufs=1))
            natf = ctx.enter_context(tc.tile_pool(name="natf", bufs=4))
            natp = ctx.enter_context(tc.tile_pool(name="natp", bufs=4))
            ktp = ctx.enter_context(tc.tile_pool(name="ktp", bufs=20))
            hp = ctx.enter_context(tc.tile_pool(name="hp", bufs=20))
            sxp = ctx.enter_context(tc.tile_pool(name="sxp", bufs=6))
            ps_t = ctx.enter_context(tc.tile_pool(name="ps_t", bufs=2, space="PSUM"))
            ps_x = ctx.enter_context(tc.tile_pool(name="ps_x", bufs=1, space="PSUM"))
            ps_12 = ctx.enter_context(tc.tile_pool(name="ps_12", bufs=2, space="PSUM"))
            ps_s = ctx.enter_context(tc.tile_pool(name="ps_s", bufs=2, space="PSUM"))
            ps_o = ctx.enter_context(tc.tile_pool(name="ps_o", bufs=1, space="PSUM"))

            # ---------------- constants / weights prep ----------------
            ident = cst.tile([128, 128], BF16, tag="ident")
            masks.make_identity(nc, ident[:])
            identf = cst.tile([128, 128], FP32, tag="identf")
            masks.make_identity(nc, identf[:])

            iota_i = cst.tile([128, L], I32, tag="iota_i")
            nc.gpsimd.iota(iota_i[:], pattern=[[1, L]], base=0, channel_multiplier=0)
            iota_f = cst.tile([128, L], FP32, tag="iota_f")
            nc.vector.tensor_copy(iota_f[:], iota_i[:])
            negbig = cst.tile([128, L], FP32, tag="negbig")
            nc.gpsimd.memset(negbig[:], NEG_BIG)

            # W1 blocks: Wa=W1[0:D], Wb=[D:2D], Wc=[2D:3D], Wp=[3D:4D]
            W1v = W1_d.rearrange("(c d) a -> c d a", c=4)
            Wa = cst.tile([128, A], FP32, tag="Wa")
            nc.sync.dma_start(Wa[:], W1v[0])
            Wb = cst.tile([128, A], FP32, tag="Wb")
            nc.sync.dma_start(Wb[:], W1v[1])
            Wc = cst.tile([128, A], FP32, tag="Wc")
            nc.sync.dma_start(Wc[:], W1v[2])
            Wp = cst.tile([128, A], FP32, tag="Wp")
            nc.sync.dma_start(Wp[:], W1v[3])
            Wq_f = cst.tile([128, A], FP32, tag="Wq_f")
            nc.vector.tensor_tensor(Wq_f[:], Wa[:], Wc[:], op=ALU.add)
            Wq_bf = cst.tile([128, A], BF16, tag="Wq_bf")
            nc.vector.tensor_copy(Wq_bf[:], Wq_f[:])
            Wk_f = cst.tile([128, A], FP32, tag="Wk_f")
            nc.vector.tensor_tensor(Wk_f[:], Wb[:], Wc[:], op=ALU.subtract)

            # W2 block-diagonal [128, 128] = diag(W2, W2), bf16
            W2f = cst.tile([128, 128], FP32, tag="W2f")
            nc.gpsimd.memset(W2f[:], 0.0)
            nc.sync.dma_start(W2f[0:64, 0:64], W2_d[:])
            nc.sync.dma_start(W2f[64:128, 64:128], W2_d[:])
            W2bf = cst.tile([128, 128], BF16, tag="W2bf")
            nc.vector.tensor_copy(W2bf[:], W2f[:])

            # W3 block-diagonal [128, 2]
            w3f = cst.tile([128, 2], FP32, tag="w3f")
            nc.gpsimd.memset(w3f[:], 0.0)
            nc.sync.dma_start(w3f[0:64, 0:1], W3_d[:])
            nc.sync.dma_start(w3f[64:128, 1:2], W3_d[:])
            w3diag = cst.tile([128, 2], BF16, tag="w3diag")
            nc.vector.tensor_copy(w3diag[:], w3f[:])

            # bias cols duplicated on both halves
            b1_dup = cst.tile([128, 1], FP32, tag="b1_dup")
            nc.sync.dma_start(b1_dup[0:64, :], b1_d[:, None])
            nc.sync.dma_start(b1_dup[64:128, :], b1_d[:, None])
            b2_dup = cst.tile([128, 1], FP32, tag="b2_dup")
            nc.sync.dma_start(b2_dup[0:64, :], b2_d[:, None])
            nc.sync.dma_start(b2_dup[64:128, :], b2_d[:, None])

            # lens slot columns [128, 2*FILLS]: col 2f+s row 32j+r = lens[16f+8s+2j+r]
            lens_sc = cst.tile([128, 2 * FILLS], FP32, tag="lens_sc")
            nc.gpsimd.memset(lens_sc[:], 1.0)
            lsrc = lens_d.rearrange("(f s j r) -> f s j r", s=2, j=4, r=2)
            ldst = lens_sc[:].rearrange("(j x r) (f s) -> j r f s x",
                                        j=4, r=2, s=2)[:, :, :, :, 0]
            for s in range(2):
                for j in range(4):
                    nc.sync.dma_start(
                        ldst[j, :, :, s],
                        lsrc[:, s, j].rearrange("f r -> r f"))
            flag_sc = cst.tile([128, 2 * FILLS], FP32, tag="flag_sc")
            nc.vector.tensor_scalar(flag_sc[:], lens_sc[:], 0.0, None,
                                    op0=ALU.is_equal)

            # ---------------- per-group (128 b) prep: qT and bias1 ----------------
            qT_f_list, bias1_list = [], []
            for g in range(2):
                q_nat = sxp.tile([128, 128], FP32, tag="q_nat")
                nc.sync.dma_start(q_nat[:], query_d[g * 128:(g + 1) * 128, :])
                qT_ps = ps_x.tile([128, 128], FP32, tag="tpsf")
                nc.tensor.transpose(qT_ps[:], q_nat[:], identf[:])
                qT_f = cst.tile([128, 128], FP32, tag=f"qT_f{g}")
                nc.vector.tensor_copy(qT_f[:], qT_ps[:])
                qT_bf = cst.tile([128, 128], BF16, tag=f"qT_bf{g}")
                nc.vector.tensor_copy(qT_bf[:], qT_f[:])
                qT_f_list.append(qT_f)

                b1ps = ps_12.tile([128, 64], FP32, tag="l12")
                qT_pair = qT_bf[:].rearrange("p (c r) -> p c r", r=2)
                nc.tensor.matmul(b1ps[0:64, :], Wq_bf[:], qT_pair[:, :, 0],
                                 start=True, stop=True)
                nc.tensor.matmul(b1ps[64:128, :], Wq_bf[:], qT_pair[:, :, 1],
                                 start=True, stop=True, tile_position=(0, 64))
                bias1 = cst.tile([128, 64], FP32, tag=f"bias1_{g}")
                nc.scalar.activation(bias1[:], b1ps[:], AF.Identity, bias=b1_dup[:])
                bias1_list.append(bias1)

            # ---------------- main loop (2-fill super-phases) ----------------
            def phase_A(f):
                """loads, cast, PE transposes, keysT evac, Wkp weights"""
                g = (f * 16) // 128
                b0f = f * 16
                nff = natf.tile([128, 4096], FP32, tag="natf")
                kv0 = keys_d[b0f:b0f + 16, 0:128, :].rearrange("h p d -> p h d")
                kv1 = keys_d[b0f:b0f + 16, 128:L, :].rearrange("h p d -> p h d")
                d0 = nff[:].rearrange("p (h c d) -> c p h d", h=16, c=2)
                nc.sync.dma_start(d0[0, :, 0:8], kv0[:, 0:8])
                nc.sync.dma_start(d0[0, :, 8:16], kv0[:, 8:16])
                nc.gpsimd.dma_start(d0[1, 0:L1C, 0:8], kv1[:, 0:8])
                nc.gpsimd.dma_start(d0[1, 0:L1C, 8:16], kv1[:, 8:16])
                nf = natp.tile([128, 4096], BF16, tag="natb")
                for ci in range(4):
                    nc.vector.tensor_copy(nf[:, 1024 * ci:1024 * ci + 1024],
                                          nff[:, 1024 * ci:1024 * ci + 1024])
                kT_pairs, wkp_pairs = [], []
                for p in range(8):
                    N = slot_N[f * 8 + p]
                    c0n = min(N, 128)
                    c1n = N - c0n
                    tp = ps_t.tile([128, 400], BF16, tag="tps")
                    for h in range(2):
                        hh = 2 * p + h
                        nc.tensor.transpose(
                            tp[:, N * h:N * h + c0n],
                            nf[0:c0n, 256 * hh:256 * hh + 128],
                            ident[0:c0n, 0:c0n])
                        if c1n > 0:
                            nc.tensor.transpose(
                                tp[:, N * h + 128:N * h + 128 + c1n],
                                nf[0:c1n, 256 * hh + 128:256 * hh + 256],
                                ident[0:c1n, 0:c1n])
                    kT = ktp.tile([128, 400], BF16, tag="kT")
                    if p % 4 != 3:
                        nc.scalar.activation(kT[:, 0:2 * N], tp[:, 0:2 * N],
                                             AF.Copy)
                    else:
                        nc.vector.tensor_copy(kT[:, 0:2 * N], tp[:, 0:2 * N])
                    kT_pairs.append(kT)
                    wkp = hp.tile([128, 128], BF16, tag="wkp")
                    qT_f = qT_f_list[g]
                    for h in range(2):
                        bcol = f * 16 + 2 * p + h - g * 128
                        qcol = qT_f[:, bcol:bcol + 1]
                        nc.vector.scalar_tensor_tensor(
                            wkp[:, 64 * h:64 * h + 64], Wp[:], qcol, Wk_f[:],
                            op0=ALU.mult, op1=ALU.add)
                    wkp_pairs.append(wkp)
                return nf, kT_pairs, wkp_pairs

            def phase_B(f, kT_pairs, wkp_pairs):
                """dense matmul burst: L1 -> PReLU1 -> L2 -> PReLU2 -> L3"""
                g = (f * 16) // 128
                sbank = ps_s.tile([128, 512], FP32, tag="sbank")
                h2p_cur = [None, None]
                for p in range(8):
                    N = slot_N[f * 8 + p]
                    kT = kT_pairs[p]
                    wkp = wkp_pairs[p]
                    psum1 = ps_12.tile([128, 200], FP32, tag="l12")
                    nc.tensor.matmul(psum1[0:64, 0:N], wkp[:, 0:64], kT[:, 0:N],
                                     start=True, stop=True, tile_position=(0, 0))
                    nc.tensor.matmul(psum1[64:128, 0:N], wkp[:, 64:128],
                                     kT[:, N:2 * N],
                                     start=True, stop=True, tile_position=(0, 64))
                    h1p = hp.tile([128, 200], BF16, tag="h1p")
                    pcol = f * 8 + p - g * 64
                    nc.scalar.activation(h1p[:, 0:N], psum1[:, 0:N], AF.Prelu,
                                         bias=bias1_list[g][:, pcol:pcol + 1],
                                         alpha=alpha1)
                    if p % 2 == 0:
                        ps2_t = ps_12.tile([128, 400], FP32, tag="l12")
                        h2p_t = hp.tile([128, 400], BF16, tag="h2p")
                        h2p_cur = [ps2_t, h2p_t, 0]
                    psum2, h2p, coff = h2p_cur
                    nc.tensor.matmul(psum2[:, coff:coff + N], W2bf[:],
                                     h1p[:, 0:N], start=True, stop=True)
                    if p % 2 == 1:
                        Nprev = slot_N[f * 8 + p - 1]
                        nc.scalar.activation(h2p[:, 0:Nprev + N],
                                             psum2[:, 0:Nprev + N], AF.Prelu,
                                             bias=b2_dup[:], alpha=alpha2)
                        for pp_, cc, NN in ((p - 1, 0, Nprev), (p, Nprev, N)):
                            j, s = pp_ % 4, pp_ // 4
                            nc.tensor.matmul(
                                sbank[32 * j:32 * j + 2, 200 * s:200 * s + NN],
                                w3diag[:], h2p[:, cc:cc + NN],
                                start=True, stop=True,
                                tile_position=(0, 32 * j),
                                skip_group_check=True)
                    else:
                        h2p_cur[2] = N
                return sbank

            def phase_C(f, nf, sbank):
                """softmax, att dump, attT transposes, output matmuls, out dump"""
                obank = ps_o.tile([128, 512], FP32, tag="obank")
                e_sb = sxp.tile([128, 400], FP32, tag="e_sb")
                Zc = sxp.tile([128, 2], FP32, tag="Zc")
                rZc = sxp.tile([128, 2], FP32, tag="rZc")
                for s in range(2):
                    half = sbank[:, 200 * s:200 * s + 200]
                    minv = sxp.tile([128, L], U8, tag="minv")
                    nc.vector.tensor_scalar(minv[:], iota_f[:],
                                            lens_sc[:, 2 * f + s:2 * f + s + 1],
                                            None, op0=ALU.is_ge)
                    nc.vector.copy_predicated(half, minv[:], negbig[:])
                    eh = e_sb[:, 200 * s:200 * s + 200]
                    nc.scalar.activation(eh, half, AF.Exp,
                                         accum_out=Zc[:, s:s + 1])
                    nc.vector.scalar_tensor_tensor(
                        Zc[:, s:s + 1], flag_sc[:, 2 * f + s:2 * f + s + 1],
                        200.0, Zc[:, s:s + 1], op0=ALU.mult, op1=ALU.add)
                    nc.vector.reciprocal(rZc[:, s:s + 1], Zc[:, s:s + 1])
                    nc.vector.tensor_scalar(eh, eh,
                                            flag_sc[:, 2 * f + s:2 * f + s + 1],
                                            rZc[:, s:s + 1],
                                            op0=ALU.add, op1=ALU.mult)
                nc.sync.dma_start(att_d[f], e_sb[:])
                e_bf = sxp.tile([128, 400], BF16, tag="e_bf")
                nc.vector.tensor_copy(e_bf[:], e_sb[:])
                attT = sxp.tile([128, 512], BF16, tag="attT")
                for s in range(2):
                    tp2f = ps_x.tile([128, 256], BF16, tag="tpsf")
                    nc.tensor.transpose(tp2f[0:128, 0:128],
                                        e_bf[:, 200 * s:200 * s + 128], ident[:])
                    nc.tensor.transpose(tp2f[0:L1C, 128:256],
                                        e_bf[:, 200 * s + 128:200 * s + 200],
                                        ident[:])
                    nc.scalar.activation(attT[:, 256 * s:256 * s + 128],
                                         tp2f[:, 0:128], AF.Copy)
                    nc.scalar.activation(attT[0:L1C, 256 * s + 128:256 * s + 256],
                                         tp2f[0:L1C, 128:256], AF.Copy)
                natv = nf[:].rearrange("q (h c d) -> q h c d", h=16, c=2)
                for p in range(8):
                    N = slot_N[f * 8 + p]
                    c0n = min(N, 128)
                    c1n = N - c0n
                    j, s = p % 4, p // 4
                    colb = 256 * s + 32 * j
                    oslot = obank[32 * j:32 * j + 2, 256 * s:256 * s + 256]
                    nc.tensor.matmul(oslot, attT[0:c0n, colb:colb + 2],
                                     natv[0:c0n, 2 * p:2 * p + 2, 0, :],
                                     start=True, stop=(c1n == 0),
                                     tile_position=(0, 32 * j),
                                     skip_group_check=True)
                    if c1n > 0:
                        nc.tensor.matmul(oslot, attT[0:c1n, colb + 128:colb + 130],
                                         natv[0:c1n, 2 * p:2 * p + 2, 1, :],
                                         start=False, stop=True,
                                         tile_position=(0, 32 * j),
                                         skip_group_check=True)
                o_sb = sxp.tile([128, 512], FP32, tag="o_sb")
                nc.vector.tensor_copy(o_sb[:], obank[:])
                nc.gpsimd.dma_start(out_d[f], o_sb[:])

            for sf in range(FILLS // 2):
                f0, f1 = 2 * sf, 2 * sf + 1
                st0 = phase_A(f0)
                st1 = phase_A(f1)
                with tc.high_priority(offset=200):
                    sb0 = phase_B(f0, st0[1], st0[2])
                    sb1 = phase_B(f1, st1[1], st1[2])
                phase_C(f0, st0[0], sb0)
                phase_C(f1, st1[0], sb1)

    nc.compile()
    return nc


_GRAPH_CACHE = {}


def kernel(**inputs):
    query = np.asarray(inputs["query"], np.float32)
    keys = np.asarray(inputs["keys"], np.float32)
    keys_length = np.asarray(inputs["keys_length"])
    W1 = np.asarray(inputs["W1"], np.float32)
    b1 = np.asarray(inputs["b1"], np.float32)
    a1 = float(np.asarray(inputs["a1"]).reshape(-1)[0])
    W2 = np.asarray(inputs["W2"], np.float32)
    b2 = np.asarray(inputs["b2"], np.float32)
    a2 = float(np.asarray(inputs["a2"]).reshape(-1)[0])
    W3 = np.asarray(inputs["W3"], np.float32)

    lens_i = keys_length.astype(np.int64)
    # sort each core's rows by length descending; device slots see sorted rows
    perms = []
    sorted_lens = np.empty((N_CORES, BC), np.int64)
    for c in range(N_CORES):
        sl = lens_i[c * BC:(c + 1) * BC]
        perm = np.argsort(-sl, kind="stable")
        perms.append(perm)
        sorted_lens[c] = sl[perm]
    # per pair-slot compiled width: max len across cores in the slot (both rows),
    # rounded up to 8; any zero-length row in the slot forces N=200 (uniform att)
    pair_lens = sorted_lens.reshape(N_CORES, 128, 2)
    slot_N = pair_lens.max(axis=(0, 2))
    has_zero = (pair_lens.min(axis=(0, 2)) == 0)
    slot_N = np.minimum(200, np.maximum(8, ((slot_N + 7) // 8) * 8))
    slot_N[has_zero] = 200
    slot_N = tuple(int(x) for x in slot_N)

    key = (a1, a2, slot_N)
    if key not in _GRAPH_CACHE:
        _GRAPH_CACHE[key] = build_graph(a1, a2, slot_N)
    nc = _GRAPH_CACHE[key]

    lens_f = keys_length.astype(np.float32)
    in_maps = []
    for c in range(N_CORES):
        sl = slice(c * BC, (c + 1) * BC)
        pm = perms[c]
        in_maps.append({
            "keys": np.ascontiguousarray(keys[sl][pm]),
            "query": np.ascontiguousarray(query[sl][pm]),
            "lens": np.ascontiguousarray(lens_f[sl][pm]),
            "W1": W1, "b1": b1, "W2": W2, "b2": b2, "W3": W3,
        })
    res = bass_utils.run_bass_kernel_spmd(
        nc, in_maps, core_ids=list(range(N_CORES)),
        trace=bool(int(os.environ.get("KERNEL_TRACE", "0"))))
    kernel.last_exec_time_ns = res.exec_time_ns
    # reindex raw slot dumps: att[16f+8s+2j+r] = att_hw[f, 32j+r, 200s:+200]
    #                         out[16f+4so+jo]  = out_hw[f, 32jo, 128so:+128]
    att = np.empty((B, L), np.float32)
    out = np.empty((B, D), np.float32)
    ii = np.arange(16)
    arow = 32 * ((ii % 8) // 2) + (ii % 2)          # i = 8s+2j+r
    acol = 200 * (ii // 8)
    # out: i = 2p+r, p=4s+j -> row 32j+r, col 256s+128r
    p_ = ii // 2
    r_ = ii % 2
    orow = 32 * (p_ % 4) + r_
    ocol = 256 * (p_ // 4) + 128 * r_
    for c in range(N_CORES):
        att_hw = res.results[c]["att"]
        out_hw = res.results[c]["out"]
        pm = perms[c]
        for f in range(FILLS):
            for i in range(16):
                b_dev = 16 * f + i
                b_orig = c * BC + pm[b_dev]
                att[b_orig] = att_hw[f, arow[i], acol[i]:acol[i] + 200]
                out[b_orig] = out_hw[f, orow[i], ocol[i]:ocol[i] + 128]
    return out, att


# revision 29
# speedup vs baseline: 1.0359x; 1.0359x over previous
"""DIN-style attention unit (dense MLP scorer + masked softmax + weighted sum)
on 8 Trainium2 NeuronCores, data-parallel over the batch dimension.

kernel(**inputs) takes the full unsharded inputs and returns (out, att):
  out: (2048, 128) f32, att: (2048, 200) f32
"""
import sys, types, os
sys.path.insert(0, "/opt/trn_rl_repo")
import numpy as np

# --- antenv.axon_hooks shim so run_bass_kernel_spmd(trace=True) can profile ---
try:
    import antenv.axon_hooks  # noqa: F401
except Exception:
    try:
        from trn_agent_boot.trn_boot import _ntff_profile_via_ctypes
        _hook = _ntff_profile_via_ctypes('/opt/axon/libaxon_pjrt.so')
    except Exception:
        _hook = None
    _m = types.ModuleType('antenv.axon_hooks')
    _m.get_axon_ntff_profile_hook = lambda: _hook
    _m.set_axon_ntff_profile_hook = lambda h: None
    sys.modules['antenv.axon_hooks'] = _m

import concourse.bass as bass
import concourse.tile as tile
from concourse import bacc, mybir
import concourse.bass_utils as bass_utils
from concourse import masks

bass_utils.upload_artifacts = lambda tmpdir: "local://" + str(tmpdir)

FP32 = mybir.dt.float32
BF16 = mybir.dt.bfloat16
F32R = mybir.dt.float32r
U8 = mybir.dt.uint8
I32 = mybir.dt.int32
AF = mybir.ActivationFunctionType
ALU = mybir.AluOpType

N_CORES = 8
B, L, D, A = 2048, 200, 128, 64
BC = B // N_CORES          # 256 batch rows per core
L1C = L - 128              # second l-chunk: 72
FILLS = BC // 16           # 16 bank-fills of 16 batch rows
NEG_BIG = -1e30


def build_graph(alpha1: float, alpha2: float, slot_N):
    nc = bacc.Bacc("TRN2", target_bir_lowering=False, debug=False,
                   num_devices=N_CORES)

    keys_d = nc.dram_tensor("keys", [BC, L, D], FP32, kind="ExternalInput").ap()
    query_d = nc.dram_tensor("query", [BC, D], FP32, kind="ExternalInput").ap()
    lens_d = nc.dram_tensor("lens", [BC], FP32, kind="ExternalInput").ap()
    W1_d = nc.dram_tensor("W1", [4 * D, A], FP32, kind="ExternalInput").ap()
    b1_d = nc.dram_tensor("b1", [A], FP32, kind="ExternalInput").ap()
    W2_d = nc.dram_tensor("W2", [A, A], FP32, kind="ExternalInput").ap()
    b2_d = nc.dram_tensor("b2", [A], FP32, kind="ExternalInput").ap()
    W3_d = nc.dram_tensor("W3", [A, 1], FP32, kind="ExternalInput").ap()
    out_d = nc.dram_tensor("out", [FILLS, 128, 512], FP32,
                           kind="ExternalOutput").ap()
    att_d = nc.dram_tensor("att", [FILLS, 128, 400], FP32,
                           kind="ExternalOutput").ap()

    with tile.TileContext(nc) as tc:
        import contextlib
        ctx = contextlib.ExitStack()
        with ctx:
            cst = ctx.enter_context(tc.tile_pool(name="cst", bufs=1))
            natf = ctx.enter_context(tc.tile_pool(name="natf", bufs=3))
            natp = ctx.enter_context(tc.tile_pool(name="natp", bufs=3))
            ktp = ctx.enter_context(tc.tile_pool(name="ktp", bufs=20))
            hp = ctx.enter_context(tc.tile_pool(name="hp", bufs=20))
            sxp = ctx.enter_context(tc.tile_pool(name="sxp", bufs=6))
            ps_t = ctx.enter_context(tc.tile_pool(name="ps_t", bufs=2, space="PSUM"))
            ps_x = ctx.enter_context(tc.tile_pool(name="ps_x", bufs=1, space="PSUM"))
            ps_12 = ctx.enter_context(tc.tile_pool(name="ps_12", bufs=3, space="PSUM"))
            ps_s = ctx.enter_context(tc.tile_pool(name="ps_s", bufs=1, space="PSUM"))
            ps_o = ctx.enter_context(tc.tile_pool(name="ps_o", bufs=1, space="PSUM"))

            # ---------------- constants / weights prep ----------------
            ident = cst.tile([128, 128], BF16, tag="ident")
            masks.make_identity(nc, ident[:])
            identf = cst.tile([128, 128], FP32, tag="identf")
            masks.make_identity(nc, identf[:])

            iota_i = cst.tile([128, L], I32, tag="iota_i")
            nc.gpsimd.iota(iota_i[:], pattern=[[1, L]], base=0, channel_multiplier=0)
            iota_f = cst.tile([128, L], FP32, tag="iota_f")
            nc.vector.tensor_copy(iota_f[:], iota_i[:])
            negbig = cst.tile([128, L], FP32, tag="negbig")
            nc.gpsimd.memset(negbig[:], NEG_BIG)

            # W1 blocks: Wa=W1[0:D], Wb=[D:2D], Wc=[2D:3D], Wp=[3D:4D]
            W1v = W1_d.rearrange("(c d) a -> c d a", c=4)
            Wa = cst.tile([128, A], FP32, tag="Wa")
            nc.sync.dma_start(Wa[:], W1v[0])
            Wb = cst.tile([128, A], FP32, tag="Wb")
            nc.sync.dma_start(Wb[:], W1v[1])
            Wc = cst.tile([128, A], FP32, tag="Wc")
            nc.sync.dma_start(Wc[:], W1v[2])
            Wp = cst.tile([128, A], FP32, tag="Wp")
            nc.sync.dma_start(Wp[:], W1v[3])
            Wq_f = cst.tile([128, A], FP32, tag="Wq_f")
            nc.vector.tensor_tensor(Wq_f[:], Wa[:], Wc[:], op=ALU.add)
            Wq_bf = cst.tile([128, A], BF16, tag="Wq_bf")
            nc.vector.tensor_copy(Wq_bf[:], Wq_f[:])
            Wk_f = cst.tile([128, A], FP32, tag="Wk_f")
            nc.vector.tensor_tensor(Wk_f[:], Wb[:], Wc[:], op=ALU.subtract)

            # W2 block-diagonal [128, 128] = diag(W2, W2), bf16
            W2f = cst.tile([128, 128], FP32, tag="W2f")
            nc.gpsimd.memset(W2f[:], 0.0)
            nc.sync.dma_start(W2f[0:64, 0:64], W2_d[:])
            nc.sync.dma_start(W2f[64:128, 64:128], W2_d[:])
            W2bf = cst.tile([128, 128], BF16, tag="W2bf")
            nc.vector.tensor_copy(W2bf[:], W2f[:])

            # W3 block-diagonal [128, 2]
            w3f = cst.tile([128, 2], FP32, tag="w3f")
            nc.gpsimd.memset(w3f[:], 0.0)
            nc.sync.dma_start(w3f[0:64, 0:1], W3_d[:])
            nc.sync.dma_start(w3f[64:128, 1:2], W3_d[:])
            w3diag = cst.tile([128, 2], BF16, tag="w3diag")
            nc.vector.tensor_copy(w3diag[:], w3f[:])

            # bias cols duplicated on both halves
            b1_dup = cst.tile([128, 1], FP32, tag="b1_dup")
            nc.sync.dma_start(b1_dup[0:64, :], b1_d[:, None])
            nc.sync.dma_start(b1_dup[64:128, :], b1_d[:, None])
            b2_dup = cst.tile([128, 1], FP32, tag="b2_dup")
            nc.sync.dma_start(b2_dup[0:64, :], b2_d[:, None])
            nc.sync.dma_start(b2_dup[64:128, :], b2_d[:, None])

            # lens slot columns [128, 2*FILLS]: col 2f+s row 32j+r = lens[16f+8s+2j+r]
            lens_sc = cst.tile([128, 2 * FILLS], FP32, tag="lens_sc")
            nc.gpsimd.memset(lens_sc[:], 1.0)
            lsrc = lens_d.rearrange("(f s j r) -> f s j r", s=2, j=4, r=2)
            ldst = lens_sc[:].rearrange("(j x r) (f s) -> j r f s x",
                                        j=4, r=2, s=2)[:, :, :, :, 0]
            for s in range(2):
                for j in range(4):
                    nc.sync.dma_start(
                        ldst[j, :, :, s],
                        lsrc[:, s, j].rearrange("f r -> r f"))
            flag_sc = cst.tile([128, 2 * FILLS], FP32, tag="flag_sc")
            nc.vector.tensor_scalar(flag_sc[:], lens_sc[:], 0.0, None,
                                    op0=ALU.is_equal)

            # ---------------- per-group (128 b) prep: qT and bias1 ----------------
            qT_f_list, bias1_list = [], []
            for g in range(2):
                q_nat = sxp.tile([128, 128], FP32, tag="q_nat")
                nc.sync.dma_start(q_nat[:], query_d[g * 128:(g + 1) * 128, :])
                qT_ps = ps_x.tile([128, 128], FP32, tag="tpsf")
                nc.tensor.transpose(qT_ps[:], q_nat[:], identf[:])
                qT_f = cst.tile([128, 128], FP32, tag=f"qT_f{g}")
                nc.vector.tensor_copy(qT_f[:], qT_ps[:])
                qT_bf = cst.tile([128, 128], BF16, tag=f"qT_bf{g}")
                nc.vector.tensor_copy(qT_bf[:], qT_f[:])
                qT_f_list.append(qT_f)

                b1ps = ps_12.tile([128, 64], FP32, tag="l12")
                qT_pair = qT_bf[:].rearrange("p (c r) -> p c r", r=2)
                nc.tensor.matmul(b1ps[0:64, :], Wq_bf[:], qT_pair[:, :, 0],
                                 start=True, stop=True)
                nc.tensor.matmul(b1ps[64:128, :], Wq_bf[:], qT_pair[:, :, 1],
                                 start=True, stop=True, tile_position=(0, 64))
                bias1 = cst.tile([128, 64], FP32, tag=f"bias1_{g}")
                nc.scalar.activation(bias1[:], b1ps[:], AF.Identity, bias=b1_dup[:])
                bias1_list.append(bias1)

            # ---------------- main loop (2-fill super-phases) ----------------
            def phase_A(f):
                """loads, cast, PE transposes, keysT evac, Wkp weights"""
                g = (f * 16) // 128
                b0f = f * 16
                nff = natf.tile([128, 4096], FP32, tag="natf")
                kv0 = keys_d[b0f:b0f + 16, 0:128, :].rearrange("h p d -> p h d")
                kv1 = keys_d[b0f:b0f + 16, 128:L, :].rearrange("h p d -> p h d")
                d0 = nff[:].rearrange("p (h c d) -> c p h d", h=16, c=2)
                nc.sync.dma_start(d0[0, :, 0:8], kv0[:, 0:8])
                nc.sync.dma_start(d0[0, :, 8:16], kv0[:, 8:16])
                nc.gpsimd.dma_start(d0[1, 0:L1C, 0:8], kv1[:, 0:8])
                nc.gpsimd.dma_start(d0[1, 0:L1C, 8:16], kv1[:, 8:16])
                nf = natp.tile([128, 4096], BF16, tag="natb")
                for ci in range(4):
                    nc.vector.tensor_copy(nf[:, 1024 * ci:1024 * ci + 1024],
                                          nff[:, 1024 * ci:1024 * ci + 1024])
                kT_pairs, wkp_pairs = [], []
                for p in range(8):
                    N = slot_N[f * 8 + p]
                    c0n = min(N, 128)
                    c1n = N - c0n
                    tp = ps_t.tile([128, 400], BF16, tag="tps")
                    for h in range(2):
                        hh = 2 * p + h
                        nc.tensor.transpose(
                            tp[:, N * h:N * h + c0n],
                            nf[0:c0n, 256 * hh:256 * hh + 128],
                            ident[0:c0n, 0:c0n])
                        if c1n > 0:
                            nc.tensor.transpose(
                                tp[:, N * h + 128:N * h + 128 + c1n],
                                nf[0:c1n, 256 * hh + 128:256 * hh + 256],
                                ident[0:c1n, 0:c1n])
                    kT = ktp.tile([128, 400], BF16, tag="kT")
                    if p % 4 != 3:
                        nc.scalar.activation(kT[:, 0:2 * N], tp[:, 0:2 * N],
                                             AF.Copy)
                    else:
                        nc.vector.tensor_copy(kT[:, 0:2 * N], tp[:, 0:2 * N])
                    kT_pairs.append(kT)
                    wkp = hp.tile([128, 128], BF16, tag="wkp")
                    qT_f = qT_f_list[g]
                    for h in range(2):
                        bcol = f * 16 + 2 * p + h - g * 128
                        qcol = qT_f[:, bcol:bcol + 1]
                        nc.vector.scalar_tensor_tensor(
                            wkp[:, 64 * h:64 * h + 64], Wp[:], qcol, Wk_f[:],
                            op0=ALU.mult, op1=ALU.add)
                    wkp_pairs.append(wkp)
                return nf, kT_pairs, wkp_pairs

            def phase_B(f, kT_pairs, wkp_pairs):
                """dense matmul burst: L1 -> PReLU1 -> L2 -> PReLU2 -> L3"""
                g = (f * 16) // 128
                sbank = ps_s.tile([128, 512], FP32, tag="sbank")
                h2p_cur = [None, None]
                for p in range(8):
                    N = slot_N[f * 8 + p]
                    kT = kT_pairs[p]
                    wkp = wkp_pairs[p]
                    psum1 = ps_12.tile([128, 200], FP32, tag="l12")
                    nc.tensor.matmul(psum1[0:64, 0:N], wkp[:, 0:64], kT[:, 0:N],
                                     start=True, stop=True, tile_position=(0, 0))
                    nc.tensor.matmul(psum1[64:128, 0:N], wkp[:, 64:128],
                                     kT[:, N:2 * N],
                                     start=True, stop=True, tile_position=(0, 64))
                    h1p = hp.tile([128, 200], BF16, tag="h1p")
                    pcol = f * 8 + p - g * 64
                    nc.scalar.activation(h1p[:, 0:N], psum1[:, 0:N], AF.Prelu,
                                         bias=bias1_list[g][:, pcol:pcol + 1],
                                         alpha=alpha1)
                    if p % 2 == 0:
                        ps2_t = ps_12.tile([128, 400], FP32, tag="l12")
                        h2p_t = hp.tile([128, 400], BF16, tag="h2p")
                        h2p_cur = [ps2_t, h2p_t, 0]
                    psum2, h2p, coff = h2p_cur
                    nc.tensor.matmul(psum2[:, coff:coff + N], W2bf[:],
                                     h1p[:, 0:N], start=True, stop=True)
                    if p % 2 == 1:
                        Nprev = slot_N[f * 8 + p - 1]
                        nc.scalar.activation(h2p[:, 0:Nprev + N],
                                             psum2[:, 0:Nprev + N], AF.Prelu,
                                             bias=b2_dup[:], alpha=alpha2)
                        for pp_, cc, NN in ((p - 1, 0, Nprev), (p, Nprev, N)):
                            j, s = pp_ % 4, pp_ // 4
                            nc.tensor.matmul(
                                sbank[32 * j:32 * j + 2, 200 * s:200 * s + NN],
                                w3diag[:], h2p[:, cc:cc + NN],
                                start=True, stop=True,
                                tile_position=(0, 32 * j),
                                skip_group_check=True)
                    else:
                        h2p_cur[2] = N
                return sbank

            def phase_C(f, nf, sbank):
                """softmax, att dump, attT transposes, output matmuls, out dump"""
                obank = ps_o.tile([128, 512], FP32, tag="obank")
                e_sb = sxp.tile([128, 400], FP32, tag="e_sb")
                Zc = sxp.tile([128, 2], FP32, tag="Zc")
                rZc = sxp.tile([128, 2], FP32, tag="rZc")
                for s in range(2):
                    half = sbank[:, 200 * s:200 * s + 200]
                    minv = sxp.tile([128, L], U8, tag="minv")
                    nc.vector.tensor_scalar(minv[:], iota_f[:],
                                            lens_sc[:, 2 * f + s:2 * f + s + 1],
                                            None, op0=ALU.is_ge)
                    nc.vector.copy_predicated(half, minv[:], negbig[:])
                    eh = e_sb[:, 200 * s:200 * s + 200]
                    nc.scalar.activation(eh, half, AF.Exp,
                                         accum_out=Zc[:, s:s + 1])
                    nc.vector.scalar_tensor_tensor(
                        Zc[:, s:s + 1], flag_sc[:, 2 * f + s:2 * f + s + 1],
                        200.0, Zc[:, s:s + 1], op0=ALU.mult, op1=ALU.add)
                    nc.vector.reciprocal(rZc[:, s:s + 1], Zc[:, s:s + 1])
                    nc.vector.tensor_scalar(eh, eh,
                                            flag_sc[:, 2 * f + s:2 * f + s + 1],
                                            rZc[:, s:s + 1],
                                            op0=ALU.add, op1=ALU.mult)
                nc.sync.dma_start(att_d[f], e_sb[:])
                e_bf = sxp.tile([128, 400], BF16, tag="e_bf")
                nc.vector.tensor_copy(e_bf[:], e_sb[:])
                attT = sxp.tile([128, 512], BF16, tag="attT")
                for s in range(2):
                    tp2f = ps_x.tile([128, 256], BF16, tag="tpsf")
                    nc.tensor.transpose(tp2f[0:128, 0:128],
                                        e_bf[:, 200 * s:200 * s + 128], ident[:])
                    nc.tensor.transpose(tp2f[0:L1C, 128:256],
                                        e_bf[:, 200 * s + 128:200 * s + 200],
                                        ident[:])
                    nc.scalar.activation(attT[:, 256 * s:256 * s + 128],
                                         tp2f[:, 0:128], AF.Copy)
                    nc.scalar.activation(attT[0:L1C, 256 * s + 128:256 * s + 256],
                                         tp2f[0:L1C, 128:256], AF.Copy)
                natv = nf[:].rearrange("q (h c d) -> q h c d", h=16, c=2)
                for p in range(8):
                    N = slot_N[f * 8 + p]
                    c0n = min(N, 128)
                    c1n = N - c0n
                    j, s = p % 4, p // 4
                    colb = 256 * s + 32 * j
                    oslot = obank[32 * j:32 * j + 2, 256 * s:256 * s + 256]
                    nc.tensor.matmul(oslot, attT[0:c0n, colb:colb + 2],
                                     natv[0:c0n, 2 * p:2 * p + 2, 0, :],
                                     start=True, stop=(c1n == 0),
                                     tile_position=(0, 32 * j),
                                     skip_group_check=True)
                    if c1n > 0:
                        nc.tensor.matmul(oslot, attT[0:c1n, colb + 128:colb + 130],
                                         natv[0:c1n, 2 * p:2 * p + 2, 1, :],
                                         start=False, stop=True,
                                         tile_position=(0, 32 * j),
                                         skip_group_check=True)
                o_sb = sxp.tile([128, 512], FP32, tag="o_sb")
                nc.vector.tensor_copy(o_sb[:], obank[:])
                nc.gpsimd.dma_start(out_d[f], o_sb[:])

            for sf in range(FILLS // 2):
                f0, f1 = 2 * sf, 2 * sf + 1
                st0 = phase_A(f0)
                st1 = phase_A(f1)
                with tc.high_priority(offset=200):
                    sb0 = phase_B(f0, st0[1], st0[2])
                    sb1 = phase_B(f1, st1[1], st1[2])
                phase_C(f0, st0[0], sb0)
                phase_C(f1, st1[0], sb1)

    nc.compile()
    return nc


_GRAPH_CACHE = {}


def kernel(**inputs):
    query = np.asarray(inputs["query"], np.float32)
    keys = np.asarray(inputs["keys"], np.float32)
    keys_length = np.asarray(inputs["keys_length"])
    W1 = np.asarray(inputs["W1"], np.float32)
    b1 = np.asarray(inputs["b1"], np.float32)
    a1 = float(np.asarray(inputs["a1"]).reshape(-1)[0])
    W2 = np.asarray(inputs["W2"], np.float32)
    b2 = np.asarray(inputs["b2"], np.float32)
    a2 = float(np.asarray(inputs["a2"]).reshape(-1)[0])
    W3 = np.asarray(inputs["W3"], np.float32)

    lens_i = keys_length.astype(np.int64)
    # sort each core's rows by length descending; device slots see sorted rows
    perms = []
    sorted_lens = np.empty((N_CORES, BC), np.int64)
    for c in range(N_CORES):
        sl = lens_i[c * BC:(c + 1) * BC]
        perm = np.argsort(-sl, kind="stable")
        perms.append(perm)
        sorted_lens[c] = sl[perm]
    # per pair-slot compiled width: max len across cores in the slot (both rows),
    # rounded up to 8; any zero-length row in the slot forces N=200 (uniform att)
    pair_lens = sorted_lens.reshape(N_CORES, 128, 2)
    slot_N = pair_lens.max(axis=(0, 2))
    has_zero = (pair_lens.min(axis=(0, 2)) == 0)
    slot_N = np.minimum(200, np.maximum(8, ((slot_N + 7) // 8) * 8))
    slot_N[has_zero] = 200
    slot_N = tuple(int(x) for x in slot_N)

    key = (a1, a2, slot_N)
    if key not in _GRAPH_CACHE:
        _GRAPH_CACHE[key] = build_graph(a1, a2, slot_N)
    nc = _GRAPH_CACHE[key]

    lens_f = keys_length.astype(np.float32)
    in_maps = []
    for c in range(N_CORES):
        sl = slice(c * BC, (c + 1) * BC)
        pm = perms[c]
        in_maps.append({
            "keys": np.ascontiguousarray(keys[sl][pm]),
            "query": np.ascontiguousarray(query[sl][pm]),
            "lens": np.ascontiguousarray(lens_f[sl][pm]),
            "W1": W1, "b1": b1, "W2": W2, "b2": b2, "W3": W3,
        })
    res = bass_utils.run_bass_kernel_spmd(
        nc, in_maps, core_ids=list(range(N_CORES)),
        trace=bool(int(os.environ.get("KERNEL_TRACE", "0"))))
    kernel.last_exec_time_ns = res.exec_time_ns
    # reindex raw slot dumps: att[16f+8s+2j+r] = att_hw[f, 32j+r, 200s:+200]
    #                         out[16f+4so+jo]  = out_hw[f, 32jo, 128so:+128]
    att = np.empty((B, L), np.float32)
    out = np.empty((B, D), np.float32)
    ii = np.arange(16)
    arow = 32 * ((ii % 8) // 2) + (ii % 2)          # i = 8s+2j+r
    acol = 200 * (ii // 8)
    # out: i = 2p+r, p=4s+j -> row 32j+r, col 256s+128r
    p_ = ii // 2
    r_ = ii % 2
    orow = 32 * (p_ % 4) + r_
    ocol = 256 * (p_ // 4) + 128 * r_
    for c in range(N_CORES):
        att_hw = res.results[c]["att"]
        out_hw = res.results[c]["out"]
        pm = perms[c]
        for f in range(FILLS):
            for i in range(16):
                b_dev = 16 * f + i
                b_orig = c * BC + pm[b_dev]
                att[b_orig] = att_hw[f, arow[i], acol[i]:acol[i] + 200]
                out[b_orig] = out_hw[f, orow[i], ocol[i]:ocol[i] + 128]
    return out, att
